# revision 16
# baseline (speedup 1.0000x reference)
"""Causal multi-head attention block (B=4, S=1024, E=1024, H=16, D=64) on 8 TRN2 cores.

Sharding: data-parallel over batch (4) x tensor-parallel over heads (2 groups of 8).
Core i handles batch i//2, head-group i%2. Each core computes its partial output
projection (row-parallel W_proj); the host sums the two TP partials per batch and
applies the (exact) bias corrections.

Device-side math per core (bf16 compute, f32 accumulate):
  qT = (Wq_g)^T x^T [+ bq_g]         [512, 1024]  (head-major rows h*64+d)
  kT = (Wk_g)^T x^T [+ bk_g]         [512, 1024]
  v  = x Wv_g                        [1024, 512]  (+ ones column per head -> denominator)
  For each head h: PT[sk, sq] = exp((kT_h^T qT_h)/8) * causal_mask (lower blocks only)
  o2T_h[d, sq] = sum_sk v_h[sk, d] * PT[sk, sq];  denom[sq] = ones-row (partition 0)
  o2T_h /= denom  (softmax normalize; no max subtraction -- logits are O(1))
  out_partial = o2T^T Wp_g           [1024, 1024]  (written bf16)
Host: out[b] = out_partial[2b] + out_partial[2b+1] + (bv_0 Wp_0 + bv_1 Wp_1 + b_proj)
(the v-bias term is exact because softmax rows sum to 1).

Perf structure (vs the 148us baseline):
  - All inputs are host-pre-arranged so each DMA lands with one fat (2-16KB)
    descriptor per partition; the critical first tiles (wq jt0, wk jt0, xT kt0)
    are separate first-issued transfers spread over the THREE dma issue rings
    (sync=HWDGE/SP, scalar=HWDGE/ACT, gpsimd=SWDGE) so the first matmul starts
    ~3us in instead of ~18us.
  - The PE instruction stream is an explicit weave: between the K=64 QK^T steps
    of each head pair (which lockstep with the Scalar-engine exps through the
    2-buf PSUM pool), full-array 128x128x512 filler units (QKV chains, v
    chains, previous pairs' PV, output projection) keep the PE busy and the
    HAM clock warm.
  - Output is written bf16 (halves output DMA), one DMA per 128-row block,
    issues alternating sync/gpsimd; PSUM evacuation alternates Vector/Scalar.
"""

import numpy as np
import ml_dtypes

import concourse.bass as bass
import concourse.tile as tile
from concourse import bacc, mybir
from concourse.bass_utils import run_bass_kernel_spmd
from concourse.masks import make_upper_triangular

BF16 = mybir.dt.bfloat16
F32 = mybir.dt.float32

B, S, E = 4, 1024, 1024
H_TOT, D = 16, 64
NCORES = 8
HL = 8            # heads per core
JL = HL * D       # 512 local qkv dim
P = 128
ET = E // P       # 8 k-tiles over embed dim
JT = JL // P      # 4 partition-tiles over local qkv dim

_NC_CACHE = {}


def build_nc(qk_bias: bool):
    nc = bacc.Bacc()

    # DRAM layouts are host-pre-arranged so that for every transfer each SBUF
    # partition's data is one contiguous DRAM run (fat descriptors):
    #   xT : [p, kt, s]   wq/wk : [p, jt, kt, j]   wv : [p, kt, j]   wp : [p, jt, e]
    xT = nc.declare_dram_parameter("xT", [P, ET * S], BF16, isOutput=False)
    wq = nc.declare_dram_parameter("wq", [P, JT * ET * P], BF16, isOutput=False)
    wk = nc.declare_dram_parameter("wk", [P, JT * ET * P], BF16, isOutput=False)
    wv = nc.declare_dram_parameter("wv", [P, ET * JL], BF16, isOutput=False)
    wp = nc.declare_dram_parameter("wp", [P, JT * E], BF16, isOutput=False)
    if qk_bias:
        bqk = nc.declare_dram_parameter("bqk", [P, 2 * JT], F32, isOutput=False)
    out = nc.declare_dram_parameter("out", [S, E], BF16, isOutput=True)

    with tile.TileContext(nc) as tc:
        with (
            tc.tile_pool(name="singles", bufs=1) as singles,
            tc.tile_pool(name="pt", bufs=6) as pt_pool,
            tc.tile_pool(name="rec", bufs=2) as rec_pool,
            tc.tile_pool(name="bc", bufs=2) as bc_pool,
            tc.tile_pool(name="outst", bufs=3) as out_pool,
            tc.tile_pool(name="ps_mm", bufs=2, space="PSUM") as ps_mm,
            tc.tile_pool(name="ps_l", bufs=2, space="PSUM") as ps_l,
            tc.tile_pool(name="ps_o", bufs=2, space="PSUM") as ps_o,
        ):
            xT_sb = singles.tile([P, ET, S], BF16)
            wq_sb = singles.tile([P, JT, ET, P], BF16)
            wk_sb = singles.tile([P, JT, ET, P], BF16)
            wv_sb = singles.tile([P, ET, JL], BF16)
            wp_sb = singles.tile([P, JT, E], BF16)

            # critical tiles first, one transfer per ring, then the bulk
            nc.scalar.dma_start(out=wq_sb[:, 0], in_=wq[:, 0:ET * P])
            nc.sync.dma_start(out=xT_sb[:, 0:4], in_=xT[:, 0:4 * S])
            nc.gpsimd.dma_start(out=wk_sb[:, 0], in_=wk[:, 0:ET * P])
            nc.scalar.dma_start(out=xT_sb[:, 4:8], in_=xT[:, 4 * S:])
            nc.gpsimd.dma_start(out=wk_sb[:, 1:4], in_=wk[:, ET * P:])
            nc.scalar.dma_start(out=wq_sb[:, 1:4], in_=wq[:, ET * P:])
            nc.sync.dma_start(out=wv_sb[:], in_=wv[:, :])
            nc.sync.dma_start(out=wp_sb[:], in_=wp[:, :])
            if qk_bias:
                bqk_sb = singles.tile([P, 2, JT], F32)
                nc.gpsimd.dma_start(out=bqk_sb[:], in_=bqk[:, :])

            # causal keep-mask for diagonal PT blocks: 1 where sq >= sk else 0
            mask_sb = singles.tile([P, P], BF16)
            make_upper_triangular(nc, mask_sb[:], val=1.0, diag=True)

            qT_sb = singles.tile([P, JT, S], BF16)   # row j = h*64+d, head-major
            kT_sb = singles.tile([P, JT, S], BF16)
            o2T_sb = singles.tile([P, JT, S], BF16)  # normalized attn out, same rows
            # [sk_p, sk_tile, head, d|ones] -- ones column per head gives the
            # softmax denominator as PSUM row 64 of the PV output
            vaug_sb = singles.tile([P, ET, HL, D + 1], BF16)
            nc.vector.memset(vaug_sb[:, :, :, D:D + 1], 1.0)

            # ---- emission units (each ~8 full-array matmuls + evacuation) ----
            def chain_unit(w_sb, dst, jt, nb, bias_ap):
                ps = ps_mm.tile([P, 512], F32, tag="mm", name=f"mm_{id(w_sb)}_{jt}_{nb}")
                for kt in range(ET):
                    nc.tensor.matmul(
                        ps[:],
                        lhsT=w_sb[:, jt, kt, :],
                        rhs=xT_sb[:, kt, nb * 512:(nb + 1) * 512],
                        start=(kt == 0), stop=(kt == ET - 1),
                    )
                if bias_ap is None:
                    nc.vector.tensor_copy(
                        out=dst[:, jt, nb * 512:(nb + 1) * 512], in_=ps[:])
                else:
                    nc.vector.tensor_scalar_add(
                        dst[:, jt, nb * 512:(nb + 1) * 512], ps[:], bias_ap)

            def v_unit(st):
                ps = ps_mm.tile([P, 512], F32, tag="mm", name=f"mmv_{st}")
                for kt in range(ET):
                    nc.tensor.matmul(
                        ps[:],
                        lhsT=xT_sb[:, kt, st * P:(st + 1) * P],
                        rhs=wv_sb[:, kt, :],
                        start=(kt == 0), stop=(kt == ET - 1),
                    )
                nc.vector.tensor_copy(
                    out=vaug_sb[:, st, :, 0:D],
                    in_=ps[:].rearrange("p (h d) -> p h d", h=HL),
                )

            def pv_unit(h, pT, sqb, pool=None):
                jt0, po = h // 2, (h % 2) * 64
                c0, c1 = sqb * 512, (sqb + 1) * 512
                pl = ps_o if pool is None else pool
                pso = pl.tile([P, 512], F32,
                              tag=("pso" if pl is ps_o else "mm"),
                              name=f"pso_{h}_{sqb}")
                ts = [t for t in range(ET) if t * P < c1]
                for i, t in enumerate(ts):
                    s0 = max(t * P, c0)
                    nc.tensor.matmul(
                        pso[0:D + 1, s0 - c0:512],
                        lhsT=vaug_sb[:, t, h, :],
                        rhs=pT[:, t, s0:c1],
                        start=(i == 0), stop=(i == len(ts) - 1),
                        skip_group_check=True,
                    )
                # normalize: o2T_h[:, c0:c1] = pso[:64] / pso[64]
                rec = rec_pool.tile([P, 512], F32, tag="rec", name=f"rec_{h}_{sqb}")
                nc.vector.tensor_copy(out=rec[:1, :], in_=pso[D:D + 1, :])
                nc.vector.reciprocal_approx_fast(out=rec[:1, :], in_=rec[:1, :])
                bcst = bc_pool.tile([P, 512], F32, tag="bc", name=f"bc_{h}_{sqb}")
                nc.gpsimd.partition_broadcast(bcst[:64, :], rec[:1, :])
                nc.vector.tensor_mul(
                    out=o2T_sb[po:po + 64, jt0, c0:c1],
                    in0=pso[0:D, :], in1=bcst[:64, :],
                )

            def proj_pair(st, split_dma=False):
                ob = out_pool.tile([P, E], BF16, tag="ob", name=f"ob_{st}")
                for eb in range(2):
                    psf = ps_mm.tile([P, 512], F32, tag="mm", name=f"mmp_{st}_{eb}")
                    for kt in range(JT):
                        nc.tensor.matmul(
                            psf[:],
                            lhsT=o2T_sb[:, kt, st * P:(st + 1) * P],
                            rhs=wp_sb[:, kt, eb * 512:(eb + 1) * 512],
                            start=(kt == 0), stop=(kt == JT - 1),
                        )
                    if eb == 0:
                        nc.vector.tensor_copy(out=ob[:, 0:512], in_=psf[:])
                        if split_dma:
                            nc.sync.dma_start(
                                out=out[st * P:(st + 1) * P, 0:512],
                                in_=ob[:, 0:512])
                    else:
                        nc.scalar.copy(out=ob[:, 512:1024], in_=psf[:])
                if split_dma:
                    nc.gpsimd.dma_start(
                        out=out[st * P:(st + 1) * P, 512:1024], in_=ob[:, 512:1024])
                else:
                    eng = nc.sync if st % 2 == 0 else nc.gpsimd
                    eng.dma_start(out=out[st * P:(st + 1) * P, :], in_=ob[:])

            def pair_views(p):
                views = []
                for hh in (2 * p, 2 * p + 1):
                    po = (hh % 2) * 64
                    views.append((
                        qT_sb[po:po + 64, p, :],
                        kT_sb[po:po + 64, p, :],
                        pt_pool.tile([P, ET, S], BF16, tag="pt", name=f"pt_{hh}"),
                    ))
                return views

            def qk_t(views, t):
                lo = t * P
                psls = [ps_l.tile([P, S], F32, tag="psl", name=f"psl_{t}_{j}") for j in range(2)]
                for cb in range(2):
                    c0, c1 = cb * 512, (cb + 1) * 512
                    s0 = max(lo, c0)
                    if s0 >= c1:
                        continue
                    # back-to-back row-half matmuls execute concurrently
                    for (qh, kh, _), psl in zip(views, psls):
                        nc.tensor.matmul(
                            psl[:, s0:c1],
                            lhsT=kh[:, lo:lo + P],
                            rhs=qh[:, s0:c1],
                            start=True, stop=True,
                        )
                for (_, _, pT), psl in zip(views, psls):
                    nc.scalar.activation(
                        out=pT[:, t, lo:S], in_=psl[:, lo:S],
                        func=mybir.ActivationFunctionType.Exp, scale=0.125,
                    )

            def mask_half(views, half):
                # diagonal blocks t=4h..4h+3 in one strided multiply: block t
                # sits at free offset t*(S+P) in the flattened PT tile. Split
                # in halves so PV sqb0 can start right after exp t3.
                t0 = half * 4
                for _, _, pT in views:
                    diag = bass.AP(tensor=pT.tensor,
                                   offset=pT.offset + t0 * (S + P),
                                   ap=[list(pT.ap[0]), [S + P, 4], [1, P]])
                    nc.vector.tensor_mul(
                        out=diag, in0=diag,
                        in1=mask_sb[:, None, :].to_broadcast([P, 4, P]),
                    )

            bias = {}
            if qk_bias:
                for jt in range(JT):
                    bias[("q", jt)] = bqk_sb[:, 0, jt:jt + 1]
                    bias[("k", jt)] = bqk_sb[:, 1, jt:jt + 1]
            else:
                for jt in range(JT):
                    bias[("q", jt)] = None
                    bias[("k", jt)] = None

            def q_unit(jt, nb):
                return lambda: chain_unit(wq_sb, qT_sb, jt, nb, bias[("q", jt)])

            def k_unit(jt, nb):
                return lambda: chain_unit(wk_sb, kT_sb, jt, nb, bias[("k", jt)])

            # ---- phase A: q0+k0 co-accumulated kt-by-kt so the consume
            # rate (8 matmuls per 256KB x-tile) matches the DMA delivery rate;
            # k0 borrows the (idle) ps_o slots ----
            psq = [ps_mm.tile([P, 512], F32, tag="mm", name=f"mmq0_{nb}")
                   for nb in range(2)]
            psk = [ps_o.tile([P, 512], F32, tag="pso", name=f"mmk0_{nb}")
                   for nb in range(2)]
            for kt in range(ET):
                for pss, w_sb in ((psq, wq_sb), (psk, wk_sb)):
                    for nb in range(2):
                        nc.tensor.matmul(
                            pss[nb][:],
                            lhsT=w_sb[:, 0, kt, :],
                            rhs=xT_sb[:, kt, nb * 512:(nb + 1) * 512],
                            start=(kt == 0), stop=(kt == ET - 1),
                        )
            for dst, pss, key in ((qT_sb, psq, ("q", 0)), (kT_sb, psk, ("k", 0))):
                for nb in range(2):
                    if bias[key] is None:
                        nc.vector.tensor_copy(
                            out=dst[:, 0, nb * 512:(nb + 1) * 512], in_=pss[nb][:])
                    else:
                        nc.vector.tensor_scalar_add(
                            dst[:, 0, nb * 512:(nb + 1) * 512], pss[nb][:], bias[key])

            # ---- attention weave: fillers between QK t-steps; each pair's
            # sqb0 PV runs inside its own window (mask halves), sqb1 early in
            # the next window ----
            def pv(h, views, sqb, pool=None):
                return lambda: pv_unit(h, views[h % 2][2], sqb, pool)

            views0 = pair_views(0)
            f0 = {0: [lambda: v_unit(0)], 1: [lambda: v_unit(1)],
                  2: [lambda: v_unit(2)], 3: [lambda: v_unit(3)],
                  4: [q_unit(1, 0), q_unit(1, 1)],
                  5: [k_unit(1, 0), k_unit(1, 1)],
                  6: [pv(0, views0, 0)], 7: [pv(1, views0, 0)]}
            for t in range(ET):
                qk_t(views0, t)
                if t == 3:
                    mask_half(views0, 0)
                for u in f0.get(t, []):
                    u()
            mask_half(views0, 1)

            views1 = pair_views(1)
            f1 = {0: [lambda: v_unit(4)], 1: [lambda: v_unit(5)],
                  2: [lambda: v_unit(6)], 3: [lambda: v_unit(7)],
                  4: [q_unit(2, 0)], 5: [q_unit(2, 1)],
                  6: [pv(2, views1, 0), k_unit(2, 0)],
                  7: [pv(3, views1, 0), k_unit(2, 1)]}
            for t in range(ET):
                qk_t(views1, t)
                if t == 3:
                    mask_half(views1, 0)
                for u in f1.get(t, []):
                    u()
            mask_half(views1, 1)

            views2 = pair_views(2)
            f2 = {0: [pv(0, views0, 1)], 1: [pv(1, views0, 1)],
                  2: [pv(2, views1, 1)], 3: [pv(3, views1, 1)],
                  4: [q_unit(3, 0)], 5: [q_unit(3, 1)],
                  6: [pv(4, views2, 0), k_unit(3, 0)],
                  7: [pv(5, views2, 0), k_unit(3, 1)]}
            for t in range(ET):
                qk_t(views2, t)
                if t == 3:
                    mask_half(views2, 0)
                for u in f2.get(t, []):
                    u()
            mask_half(views2, 1)

            views3 = pair_views(3)
            f3 = {0: [pv(4, views2, 1)], 1: [pv(5, views2, 1, ps_mm)],
                  4: [pv(6, views3, 0)], 5: [pv(7, views3, 0, ps_mm)],
                  7: [lambda: proj_pair(0)]}
            for t in range(ET):
                qk_t(views3, t)
                if t == 3:
                    mask_half(views3, 0)
                for u in f3.get(t, []):
                    u()
            mask_half(views3, 1)

            # ---- endgame: last sqb1 PVs woven into the projection ----
            pv_unit(6, views3[0][2], 1)
            proj_pair(1)
            pv_unit(7, views3[1][2], 1, ps_o)
            for st in range(2, ET):
                proj_pair(st, split_dma=(st >= ET - 2))

    nc.compile()
    return nc


def make_in_maps(x, W_attn, b_attn, W_proj, b_proj):
    bf16 = ml_dtypes.bfloat16
    x = np.asarray(x, dtype=np.float32)
    W_attn = np.asarray(W_attn, dtype=np.float32)
    b_attn = np.asarray(b_attn, dtype=np.float32)
    W_proj = np.asarray(W_proj, dtype=np.float32)
    qk_bias = bool(b_attn[:2 * E].any())
    in_maps = []
    for i in range(NCORES):
        b, g = i // 2, i % 2
        j0 = g * JL
        wq_s = W_attn[:, j0:j0 + JL]
        wk_s = W_attn[:, E + j0:E + j0 + JL]
        wv_s = W_attn[:, 2 * E + j0:2 * E + j0 + JL]
        wp_s = W_proj[j0:j0 + JL, :]
        m = {
            # [p, kt, s]: partition-contiguous x^T
            "xT": np.ascontiguousarray(
                x[b].T.reshape(ET, P, S).transpose(1, 0, 2)
            ).astype(bf16).reshape(P, ET * S),
            # [p, jt, kt, j]
            "wq": np.ascontiguousarray(
                wq_s.reshape(ET, P, JT, P).transpose(1, 2, 0, 3)
            ).astype(bf16).reshape(P, JT * ET * P),
            "wk": np.ascontiguousarray(
                wk_s.reshape(ET, P, JT, P).transpose(1, 2, 0, 3)
            ).astype(bf16).reshape(P, JT * ET * P),
            # [p, kt, j]
            "wv": np.ascontiguousarray(
                wv_s.reshape(ET, P, JL).transpose(1, 0, 2)
            ).astype(bf16).reshape(P, ET * JL),
            # [p, jt, e]
            "wp": np.ascontiguousarray(
                wp_s.reshape(JT, P, E).transpose(1, 0, 2)
            ).astype(bf16).reshape(P, JT * E),
        }
        if qk_bias:
            bq = b_attn[j0:j0 + JL].reshape(JT, P).T
            bk = b_attn[E + j0:E + j0 + JL].reshape(JT, P).T
            m["bqk"] = np.ascontiguousarray(
                np.stack([bq, bk], axis=1)).reshape(P, 2 * JT).astype(np.float32)
        in_maps.append(m)
    return in_maps


def kernel(x, W_attn, b_attn, W_proj, b_proj):
    global _NC_CACHE
    x = np.asarray(x, dtype=np.float32)
    W_attn = np.asarray(W_attn, dtype=np.float32)
    b_attn = np.asarray(b_attn, dtype=np.float32)
    W_proj = np.asarray(W_proj, dtype=np.float32)
    b_proj = np.asarray(b_proj, dtype=np.float32)

    qk_bias = bool(b_attn[:2 * E].any())
    if qk_bias not in _NC_CACHE:
        _NC_CACHE[qk_bias] = build_nc(qk_bias)
    nc = _NC_CACHE[qk_bias]

    in_maps = make_in_maps(x, W_attn, b_attn, W_proj, b_proj)
    res = run_bass_kernel_spmd(nc, in_maps, core_ids=list(range(NCORES)))

    # host unshard: sum the two head-group partials + exact bias corrections
    bias_row = b_proj.copy()
    for g in range(2):
        j0 = g * JL
        bv = b_attn[2 * E + j0:2 * E + j0 + JL].astype(np.float32)
        bias_row += bv @ W_proj[j0:j0 + JL, :].astype(np.float32)

    full = np.empty((B, S, E), np.float32)
    for b in range(B):
        full[b] = (res.results[2 * b]["out"].astype(np.float32)
                   + res.results[2 * b + 1]["out"].astype(np.float32)
                   + bias_row[None, :])
    return full


# revision 17
# speedup vs baseline: 1.0248x; 1.0248x over previous
"""Causal multi-head attention block (B=4, S=1024, E=1024, H=16, D=64) on 8 TRN2 cores.

Sharding: data-parallel over batch (4) x tensor-parallel over heads (2 groups of 8).
Core i handles batch i//2, head-group i%2. Each core computes its partial output
projection (row-parallel W_proj); the host sums the two TP partials per batch and
applies the (exact) bias corrections.

Device-side math per core (bf16 compute, f32 accumulate):
  qT = (Wq_g)^T x^T [+ bq_g]         [512, 1024]  (head-major rows h*64+d)
  kT = (Wk_g)^T x^T [+ bk_g]         [512, 1024]
  v  = x Wv_g                        [1024, 512]  (+ ones column per head -> denominator)
  For each head h: PT[sk, sq] = exp((kT_h^T qT_h)/8) * causal_mask (lower blocks only)
  o2T_h[d, sq] = sum_sk v_h[sk, d] * PT[sk, sq];  denom[sq] = ones-row (partition 0)
  o2T_h /= denom  (softmax normalize; no max subtraction -- logits are O(1))
  out_partial = o2T^T Wp_g           [1024, 1024]  (written bf16)
Host: out[b] = out_partial[2b] + out_partial[2b+1] + (bv_0 Wp_0 + bv_1 Wp_1 + b_proj)
(the v-bias term is exact because softmax rows sum to 1).

Perf structure (vs the 148us baseline):
  - All inputs are host-pre-arranged so each DMA lands with one fat (2-16KB)
    descriptor per partition; the critical first tiles (wq jt0, wk jt0, xT kt0)
    are separate first-issued transfers spread over the THREE dma issue rings
    (sync=HWDGE/SP, scalar=HWDGE/ACT, gpsimd=SWDGE) so the first matmul starts
    ~3us in instead of ~18us.
  - The PE instruction stream is an explicit weave: between the K=64 QK^T steps
    of each head pair (which lockstep with the Scalar-engine exps through the
    2-buf PSUM pool), full-array 128x128x512 filler units (QKV chains, v
    chains, previous pairs' PV, output projection) keep the PE busy and the
    HAM clock warm.
  - Output is written bf16 (halves output DMA), one DMA per 128-row block,
    issues alternating sync/gpsimd; PSUM evacuation alternates Vector/Scalar.
"""

import numpy as np
import ml_dtypes

import concourse.bass as bass
import concourse.tile as tile
from concourse import bacc, mybir
from concourse.bass_utils import run_bass_kernel_spmd
from concourse.masks import make_upper_triangular

BF16 = mybir.dt.bfloat16
F32 = mybir.dt.float32

B, S, E = 4, 1024, 1024
H_TOT, D = 16, 64
NCORES = 8
HL = 8            # heads per core
JL = HL * D       # 512 local qkv dim
P = 128
ET = E // P       # 8 k-tiles over embed dim
JT = JL // P      # 4 partition-tiles over local qkv dim

_NC_CACHE = {}


def build_nc(qk_bias: bool):
    nc = bacc.Bacc()

    # DRAM layouts are host-pre-arranged so that for every transfer each SBUF
    # partition's data is one contiguous DRAM run (fat descriptors):
    #   xT : [p, kt, s]   wq/wk : [p, jt, kt, j]   wv : [p, kt, j]   wp : [p, jt, e]
    xT = nc.declare_dram_parameter("xT", [P, ET * S], BF16, isOutput=False)
    wq = nc.declare_dram_parameter("wq", [P, JT * ET * P], BF16, isOutput=False)
    wk = nc.declare_dram_parameter("wk", [P, JT * ET * P], BF16, isOutput=False)
    wv = nc.declare_dram_parameter("wv", [P, ET * JL], BF16, isOutput=False)
    wp = nc.declare_dram_parameter("wp", [P, JT * E], BF16, isOutput=False)
    if qk_bias:
        bqk = nc.declare_dram_parameter("bqk", [P, 2 * JT], F32, isOutput=False)
    out = nc.declare_dram_parameter("out", [S, E], BF16, isOutput=True)

    with tile.TileContext(nc) as tc:
        with (
            tc.tile_pool(name="singles", bufs=1) as singles,
            tc.tile_pool(name="pt", bufs=6) as pt_pool,
            tc.tile_pool(name="rec", bufs=2) as rec_pool,
            tc.tile_pool(name="bc", bufs=2) as bc_pool,
            tc.tile_pool(name="outst", bufs=3) as out_pool,
            tc.tile_pool(name="ps_mm", bufs=2, space="PSUM") as ps_mm,
            tc.tile_pool(name="ps_l", bufs=2, space="PSUM") as ps_l,
            tc.tile_pool(name="ps_o", bufs=2, space="PSUM") as ps_o,
        ):
            xT_sb = singles.tile([P, ET, S], BF16)
            wq_sb = singles.tile([P, JT, ET, P], BF16)
            wk_sb = singles.tile([P, JT, ET, P], BF16)
            wv_sb = singles.tile([P, ET, JL], BF16)
            wp_sb = singles.tile([P, JT, E], BF16)

            # critical tiles first, one transfer per ring, then the bulk
            nc.scalar.dma_start(out=wq_sb[:, 0], in_=wq[:, 0:ET * P])
            nc.sync.dma_start(out=xT_sb[:, 0:4], in_=xT[:, 0:4 * S])
            nc.gpsimd.dma_start(out=wk_sb[:, 0], in_=wk[:, 0:ET * P])
            nc.scalar.dma_start(out=xT_sb[:, 4:8], in_=xT[:, 4 * S:])
            nc.gpsimd.dma_start(out=wk_sb[:, 1:4], in_=wk[:, ET * P:])
            nc.scalar.dma_start(out=wq_sb[:, 1:4], in_=wq[:, ET * P:])
            nc.sync.dma_start(out=wv_sb[:], in_=wv[:, :])
            nc.sync.dma_start(out=wp_sb[:], in_=wp[:, :])
            if qk_bias:
                bqk_sb = singles.tile([P, 2, JT], F32)
                nc.gpsimd.dma_start(out=bqk_sb[:], in_=bqk[:, :])

            # causal keep-mask for diagonal PT blocks: 1 where sq >= sk else 0
            mask_sb = singles.tile([P, P], BF16)
            make_upper_triangular(nc, mask_sb[:], val=1.0, diag=True)

            qT_sb = singles.tile([P, JT, S], BF16)   # row j = h*64+d, head-major
            kT_sb = singles.tile([P, JT, S], BF16)
            o2T_sb = singles.tile([P, JT, S], BF16)  # normalized attn out, same rows
            # [sk_p, sk_tile, head, d|ones] -- ones column per head gives the
            # softmax denominator as PSUM row 64 of the PV output
            vaug_sb = singles.tile([P, ET, HL, D + 1], BF16)
            nc.vector.memset(vaug_sb[:, :, :, D:D + 1], 1.0)

            # ---- emission units (each ~8 full-array matmuls + evacuation) ----
            def chain_unit(w_sb, dst, jt, nb, bias_ap):
                ps = ps_mm.tile([P, 512], F32, tag="mm", name=f"mm_{id(w_sb)}_{jt}_{nb}")
                for kt in range(ET):
                    nc.tensor.matmul(
                        ps[:],
                        lhsT=w_sb[:, jt, kt, :],
                        rhs=xT_sb[:, kt, nb * 512:(nb + 1) * 512],
                        start=(kt == 0), stop=(kt == ET - 1),
                    )
                if bias_ap is None:
                    nc.vector.tensor_copy(
                        out=dst[:, jt, nb * 512:(nb + 1) * 512], in_=ps[:])
                else:
                    nc.vector.tensor_scalar_add(
                        dst[:, jt, nb * 512:(nb + 1) * 512], ps[:], bias_ap)

            def v_unit(st):
                ps = ps_mm.tile([P, 512], F32, tag="mm", name=f"mmv_{st}")
                for kt in range(ET):
                    nc.tensor.matmul(
                        ps[:],
                        lhsT=xT_sb[:, kt, st * P:(st + 1) * P],
                        rhs=wv_sb[:, kt, :],
                        start=(kt == 0), stop=(kt == ET - 1),
                    )
                nc.vector.tensor_copy(
                    out=vaug_sb[:, st, :, 0:D],
                    in_=ps[:].rearrange("p (h d) -> p h d", h=HL),
                )

            def pv_unit(h, pT, sqb, pool=None):
                jt0, po = h // 2, (h % 2) * 64
                c0, c1 = sqb * 512, (sqb + 1) * 512
                pl = ps_o if pool is None else pool
                pso = pl.tile([P, 512], F32,
                              tag=("pso" if pl is ps_o else "mm"),
                              name=f"pso_{h}_{sqb}")
                ts = [t for t in range(ET) if t * P < c1]
                for i, t in enumerate(ts):
                    s0 = max(t * P, c0)
                    nc.tensor.matmul(
                        pso[0:D + 1, s0 - c0:512],
                        lhsT=vaug_sb[:, t, h, :],
                        rhs=pT[:, t, s0:c1],
                        start=(i == 0), stop=(i == len(ts) - 1),
                        skip_group_check=True,
                    )
                # normalize: o2T_h[:, c0:c1] = pso[:64] / pso[64]
                rec = rec_pool.tile([P, 512], F32, tag="rec", name=f"rec_{h}_{sqb}")
                nc.vector.tensor_copy(out=rec[:1, :], in_=pso[D:D + 1, :])
                nc.vector.reciprocal_approx_fast(out=rec[:1, :], in_=rec[:1, :])
                bcst = bc_pool.tile([P, 512], F32, tag="bc", name=f"bc_{h}_{sqb}")
                nc.gpsimd.partition_broadcast(bcst[:64, :], rec[:1, :])
                nc.vector.tensor_mul(
                    out=o2T_sb[po:po + 64, jt0, c0:c1],
                    in0=pso[0:D, :], in1=bcst[:64, :],
                )

            def proj_pair(st, split_dma=False):
                ob = out_pool.tile([P, E], BF16, tag="ob", name=f"ob_{st}")
                for eb in range(2):
                    psf = ps_mm.tile([P, 512], F32, tag="mm", name=f"mmp_{st}_{eb}")
                    for kt in range(JT):
                        nc.tensor.matmul(
                            psf[:],
                            lhsT=o2T_sb[:, kt, st * P:(st + 1) * P],
                            rhs=wp_sb[:, kt, eb * 512:(eb + 1) * 512],
                            start=(kt == 0), stop=(kt == JT - 1),
                        )
                    if eb == 0:
                        nc.vector.tensor_copy(out=ob[:, 0:512], in_=psf[:])
                        if split_dma:
                            nc.sync.dma_start(
                                out=out[st * P:(st + 1) * P, 0:512],
                                in_=ob[:, 0:512])
                    else:
                        nc.scalar.copy(out=ob[:, 512:1024], in_=psf[:])
                if split_dma:
                    nc.gpsimd.dma_start(
                        out=out[st * P:(st + 1) * P, 512:1024], in_=ob[:, 512:1024])
                else:
                    eng = nc.sync if st % 2 == 0 else nc.gpsimd
                    eng.dma_start(out=out[st * P:(st + 1) * P, :], in_=ob[:])

            def pair_views(p):
                views = []
                for hh in (2 * p, 2 * p + 1):
                    po = (hh % 2) * 64
                    views.append((
                        qT_sb[po:po + 64, p, :],
                        kT_sb[po:po + 64, p, :],
                        pt_pool.tile([P, ET, S], BF16, tag="pt", name=f"pt_{hh}"),
                    ))
                return views

            def qk_t(views, t):
                lo = t * P
                psls = [ps_l.tile([P, S], F32, tag="psl", name=f"psl_{t}_{j}") for j in range(2)]
                for cb in range(2):
                    c0, c1 = cb * 512, (cb + 1) * 512
                    s0 = max(lo, c0)
                    if s0 >= c1:
                        continue
                    # back-to-back row-half matmuls execute concurrently
                    for (qh, kh, _), psl in zip(views, psls):
                        nc.tensor.matmul(
                            psl[:, s0:c1],
                            lhsT=kh[:, lo:lo + P],
                            rhs=qh[:, s0:c1],
                            start=True, stop=True,
                        )
                for (_, _, pT), psl in zip(views, psls):
                    nc.scalar.activation(
                        out=pT[:, t, lo:S], in_=psl[:, lo:S],
                        func=mybir.ActivationFunctionType.Exp, scale=0.125,
                    )

            def mask_half(views, half):
                # diagonal blocks t=4h..4h+3 in one strided multiply: block t
                # sits at free offset t*(S+P) in the flattened PT tile. Split
                # in halves so PV sqb0 can start right after exp t3.
                t0 = half * 4
                for _, _, pT in views:
                    diag = bass.AP(tensor=pT.tensor,
                                   offset=pT.offset + t0 * (S + P),
                                   ap=[list(pT.ap[0]), [S + P, 4], [1, P]])
                    nc.vector.tensor_mul(
                        out=diag, in0=diag,
                        in1=mask_sb[:, None, :].to_broadcast([P, 4, P]),
                    )

            bias = {}
            if qk_bias:
                for jt in range(JT):
                    bias[("q", jt)] = bqk_sb[:, 0, jt:jt + 1]
                    bias[("k", jt)] = bqk_sb[:, 1, jt:jt + 1]
            else:
                for jt in range(JT):
                    bias[("q", jt)] = None
                    bias[("k", jt)] = None

            def q_unit(jt, nb):
                return lambda: chain_unit(wq_sb, qT_sb, jt, nb, bias[("q", jt)])

            def k_unit(jt, nb):
                return lambda: chain_unit(wk_sb, kT_sb, jt, nb, bias[("k", jt)])

            # ---- phase A: q0+k0 co-accumulated kt-by-kt so the consume
            # rate (8 matmuls per 256KB x-tile) matches the DMA delivery rate;
            # k0 borrows the (idle) ps_o slots ----
            psq = [ps_mm.tile([P, 512], F32, tag="mm", name=f"mmq0_{nb}")
                   for nb in range(2)]
            psk = [ps_o.tile([P, 512], F32, tag="pso", name=f"mmk0_{nb}")
                   for nb in range(2)]
            for kt in range(ET):
                for pss, w_sb in ((psq, wq_sb), (psk, wk_sb)):
                    for nb in range(2):
                        nc.tensor.matmul(
                            pss[nb][:],
                            lhsT=w_sb[:, 0, kt, :],
                            rhs=xT_sb[:, kt, nb * 512:(nb + 1) * 512],
                            start=(kt == 0), stop=(kt == ET - 1),
                        )
            for dst, pss, key in ((qT_sb, psq, ("q", 0)), (kT_sb, psk, ("k", 0))):
                for nb in range(2):
                    if bias[key] is None:
                        nc.vector.tensor_copy(
                            out=dst[:, 0, nb * 512:(nb + 1) * 512], in_=pss[nb][:])
                    else:
                        nc.vector.tensor_scalar_add(
                            dst[:, 0, nb * 512:(nb + 1) * 512], pss[nb][:], bias[key])

            # ---- attention weave: fillers between QK t-steps; each pair's
            # sqb0 PV runs inside its own window (mask halves), sqb1 early in
            # the next window ----
            def pv(h, views, sqb, pool=None):
                return lambda: pv_unit(h, views[h % 2][2], sqb, pool)

            views0 = pair_views(0)
            f0 = {0: [lambda: v_unit(0)], 1: [lambda: v_unit(1)],
                  2: [lambda: v_unit(2)], 3: [lambda: v_unit(3)],
                  4: [q_unit(1, 0), q_unit(1, 1)],
                  5: [k_unit(1, 0), k_unit(1, 1)],
                  6: [pv(0, views0, 0)], 7: [pv(1, views0, 0)]}
            for t in range(ET):
                qk_t(views0, t)
                if t == 3:
                    mask_half(views0, 0)
                for u in f0.get(t, []):
                    u()
            mask_half(views0, 1)

            views1 = pair_views(1)
            f1 = {0: [lambda: v_unit(4)], 1: [lambda: v_unit(5)],
                  2: [lambda: v_unit(6)], 3: [lambda: v_unit(7)],
                  4: [q_unit(2, 0)], 5: [q_unit(2, 1)],
                  6: [pv(2, views1, 0), k_unit(2, 0)],
                  7: [pv(3, views1, 0), k_unit(2, 1)]}
            for t in range(ET):
                qk_t(views1, t)
                if t == 3:
                    mask_half(views1, 0)
                for u in f1.get(t, []):
                    u()
            mask_half(views1, 1)

            views2 = pair_views(2)
            f2 = {0: [pv(0, views0, 1)], 1: [pv(1, views0, 1)],
                  2: [pv(2, views1, 1)], 3: [pv(3, views1, 1)],
                  4: [q_unit(3, 0)], 5: [q_unit(3, 1)],
                  6: [pv(4, views2, 0), k_unit(3, 0)],
                  7: [pv(5, views2, 0), k_unit(3, 1)]}
            for t in range(ET):
                qk_t(views2, t)
                if t == 3:
                    mask_half(views2, 0)
                for u in f2.get(t, []):
                    u()
            mask_half(views2, 1)

            views3 = pair_views(3)
            f3 = {0: [pv(4, views2, 1)], 1: [pv(5, views2, 1)],
                  4: [pv(6, views3, 0)], 5: [pv(7, views3, 0)],
                  7: [lambda: proj_pair(0)]}
            for t in range(ET):
                qk_t(views3, t)
                if t == 3:
                    mask_half(views3, 0)
                for u in f3.get(t, []):
                    u()
            mask_half(views3, 1)

            # ---- endgame: last sqb1 PVs woven into the projection ----
            pv_unit(6, views3[0][2], 1)
            proj_pair(1)
            pv_unit(7, views3[1][2], 1)
            for st in range(2, ET):
                proj_pair(st, split_dma=(st >= ET - 2))

    nc.compile()
    return nc


def make_in_maps(x, W_attn, b_attn, W_proj, b_proj):
    bf16 = ml_dtypes.bfloat16
    x = np.asarray(x, dtype=np.float32)
    W_attn = np.asarray(W_attn, dtype=np.float32)
    b_attn = np.asarray(b_attn, dtype=np.float32)
    W_proj = np.asarray(W_proj, dtype=np.float32)
    qk_bias = bool(b_attn[:2 * E].any())
    in_maps = []
    for i in range(NCORES):
        b, g = i // 2, i % 2
        j0 = g * JL
        wq_s = W_attn[:, j0:j0 + JL]
        wk_s = W_attn[:, E + j0:E + j0 + JL]
        wv_s = W_attn[:, 2 * E + j0:2 * E + j0 + JL]
        wp_s = W_proj[j0:j0 + JL, :]
        m = {
            # [p, kt, s]: partition-contiguous x^T
            "xT": np.ascontiguousarray(
                x[b].T.reshape(ET, P, S).transpose(1, 0, 2)
            ).astype(bf16).reshape(P, ET * S),
            # [p, jt, kt, j]
            "wq": np.ascontiguousarray(
                wq_s.reshape(ET, P, JT, P).transpose(1, 2, 0, 3)
            ).astype(bf16).reshape(P, JT * ET * P),
            "wk": np.ascontiguousarray(
                wk_s.reshape(ET, P, JT, P).transpose(1, 2, 0, 3)
            ).astype(bf16).reshape(P, JT * ET * P),
            # [p, kt, j]
            "wv": np.ascontiguousarray(
                wv_s.reshape(ET, P, JL).transpose(1, 0, 2)
            ).astype(bf16).reshape(P, ET * JL),
            # [p, jt, e]
            "wp": np.ascontiguousarray(
                wp_s.reshape(JT, P, E).transpose(1, 0, 2)
            ).astype(bf16).reshape(P, JT * E),
        }
        if qk_bias:
            bq = b_attn[j0:j0 + JL].reshape(JT, P).T
            bk = b_attn[E + j0:E + j0 + JL].reshape(JT, P).T
            m["bqk"] = np.ascontiguousarray(
                np.stack([bq, bk], axis=1)).reshape(P, 2 * JT).astype(np.float32)
        in_maps.append(m)
    return in_maps


def kernel(x, W_attn, b_attn, W_proj, b_proj):
    global _NC_CACHE
    x = np.asarray(x, dtype=np.float32)
    W_attn = np.asarray(W_attn, dtype=np.float32)
    b_attn = np.asarray(b_attn, dtype=np.float32)
    W_proj = np.asarray(W_proj, dtype=np.float32)
    b_proj = np.asarray(b_proj, dtype=np.float32)

    qk_bias = bool(b_attn[:2 * E].any())
    if qk_bias not in _NC_CACHE:
        _NC_CACHE[qk_bias] = build_nc(qk_bias)
    nc = _NC_CACHE[qk_bias]

    in_maps = make_in_maps(x, W_attn, b_attn, W_proj, b_proj)
    res = run_bass_kernel_spmd(nc, in_maps, core_ids=list(range(NCORES)))

    # host unshard: sum the two head-group partials + exact bias corrections
    bias_row = b_proj.copy()
    for g in range(2):
        j0 = g * JL
        bv = b_attn[2 * E + j0:2 * E + j0 + JL].astype(np.float32)
        bias_row += bv @ W_proj[j0:j0 + JL, :].astype(np.float32)

    full = np.empty((B, S, E), np.float32)
    for b in range(B):
        full[b] = (res.results[2 * b]["out"].astype(np.float32)
                   + res.results[2 * b + 1]["out"].astype(np.float32)
                   + bias_row[None, :])
    return full


# revision 18
# speedup vs baseline: 1.0556x; 1.0300x over previous
"""Causal multi-head attention block (B=4, S=1024, E=1024, H=16, D=64) on 8 TRN2 cores.

Sharding: data-parallel over batch (4) x tensor-parallel over heads (2 groups of 8).
Core i handles batch i//2, head-group i%2. Each core computes its partial output
projection (row-parallel W_proj); the host sums the two TP partials per batch and
applies the (exact) bias corrections.

Device-side math per core (bf16 compute, f32 accumulate):
  qT = (Wq_g)^T x^T [+ bq_g]         [512, 1024]  (head-major rows h*64+d)
  kT = (Wk_g)^T x^T [+ bk_g]         [512, 1024]
  v  = x Wv_g                        [1024, 512]  (+ ones column per head -> denominator)
  For each head h: PT[sk, sq] = exp((kT_h^T qT_h)/8) * causal_mask (lower blocks only)
  o2T_h[d, sq] = sum_sk v_h[sk, d] * PT[sk, sq];  denom[sq] = ones-row (partition 0)
  o2T_h /= denom  (softmax normalize; no max subtraction -- logits are O(1))
  out_partial = o2T^T Wp_g           [1024, 1024]  (written bf16)
Host: out[b] = out_partial[2b] + out_partial[2b+1] + (bv_0 Wp_0 + bv_1 Wp_1 + b_proj)
(the v-bias term is exact because softmax rows sum to 1).

Perf structure (vs the 148us baseline):
  - All inputs are host-pre-arranged so each DMA lands with one fat (2-16KB)
    descriptor per partition; the critical first tiles (wq jt0, wk jt0, xT kt0)
    are separate first-issued transfers spread over the THREE dma issue rings
    (sync=HWDGE/SP, scalar=HWDGE/ACT, gpsimd=SWDGE) so the first matmul starts
    ~3us in instead of ~18us.
  - The PE instruction stream is an explicit weave: between the K=64 QK^T steps
    of each head pair (which lockstep with the Scalar-engine exps through the
    2-buf PSUM pool), full-array 128x128x512 filler units (QKV chains, v
    chains, previous pairs' PV, output projection) keep the PE busy and the
    HAM clock warm.
  - Output is written bf16 (halves output DMA), one DMA per 128-row block,
    issues alternating sync/gpsimd; PSUM evacuation alternates Vector/Scalar.
"""

import numpy as np
import ml_dtypes

import concourse.bass as bass
import concourse.tile as tile
from concourse import bacc, mybir
from concourse.bass_utils import run_bass_kernel_spmd
from concourse.masks import make_upper_triangular

BF16 = mybir.dt.bfloat16
F32 = mybir.dt.float32

B, S, E = 4, 1024, 1024
H_TOT, D = 16, 64
NCORES = 8
HL = 8            # heads per core
JL = HL * D       # 512 local qkv dim
P = 128
ET = E // P       # 8 k-tiles over embed dim
JT = JL // P      # 4 partition-tiles over local qkv dim

_NC_CACHE = {}


def build_nc(qk_bias: bool):
    nc = bacc.Bacc()

    # DRAM layouts are host-pre-arranged so that for every transfer each SBUF
    # partition's data is one contiguous DRAM run (fat descriptors):
    #   xT : [p, kt, s]   wq/wk : [p, jt, kt, j]   wv : [p, kt, j]   wp : [p, jt, e]
    xT = nc.declare_dram_parameter("xT", [P, ET * S], BF16, isOutput=False)
    wq = nc.declare_dram_parameter("wq", [P, JT * ET * P], BF16, isOutput=False)
    wk = nc.declare_dram_parameter("wk", [P, JT * ET * P], BF16, isOutput=False)
    wv = nc.declare_dram_parameter("wv", [P, ET * JL], BF16, isOutput=False)
    wp = nc.declare_dram_parameter("wp", [P, JT * E], BF16, isOutput=False)
    if qk_bias:
        bqk = nc.declare_dram_parameter("bqk", [P, 2 * JT], F32, isOutput=False)
    out = nc.declare_dram_parameter("out", [S, E], BF16, isOutput=True)

    with tile.TileContext(nc) as tc:
        with (
            tc.tile_pool(name="singles", bufs=1) as singles,
            tc.tile_pool(name="pt", bufs=6) as pt_pool,
            tc.tile_pool(name="rec", bufs=2) as rec_pool,
            tc.tile_pool(name="bc", bufs=2) as bc_pool,
            tc.tile_pool(name="outst", bufs=3) as out_pool,
            tc.tile_pool(name="ps_mm", bufs=2, space="PSUM") as ps_mm,
            tc.tile_pool(name="ps_l", bufs=2, space="PSUM") as ps_l,
            tc.tile_pool(name="ps_o", bufs=2, space="PSUM") as ps_o,
        ):
            xT_sb = singles.tile([P, ET, S], BF16)
            wq_sb = singles.tile([P, JT, ET, P], BF16)
            wk_sb = singles.tile([P, JT, ET, P], BF16)
            wv_sb = singles.tile([P, ET, JL], BF16)
            wp_sb = singles.tile([P, JT, E], BF16)

            # critical tiles first, one transfer per ring, then the bulk
            nc.scalar.dma_start(out=wq_sb[:, 0], in_=wq[:, 0:ET * P])
            nc.sync.dma_start(out=xT_sb[:, 0:4], in_=xT[:, 0:4 * S])
            nc.gpsimd.dma_start(out=wk_sb[:, 0], in_=wk[:, 0:ET * P])
            nc.sync.dma_start(out=xT_sb[:, 4:8], in_=xT[:, 4 * S:])
            nc.scalar.dma_start(out=wq_sb[:, 1:4], in_=wq[:, ET * P:])
            nc.gpsimd.dma_start(out=wk_sb[:, 1:4], in_=wk[:, ET * P:])
            nc.sync.dma_start(out=wv_sb[:], in_=wv[:, :])
            nc.scalar.dma_start(out=wp_sb[:], in_=wp[:, :])
            if qk_bias:
                bqk_sb = singles.tile([P, 2, JT], F32)
                nc.gpsimd.dma_start(out=bqk_sb[:], in_=bqk[:, :])

            # causal keep-mask for diagonal PT blocks: 1 where sq >= sk else 0
            mask_sb = singles.tile([P, P], BF16)
            make_upper_triangular(nc, mask_sb[:], val=1.0, diag=True)

            qT_sb = singles.tile([P, JT, S], BF16)   # row j = h*64+d, head-major
            kT_sb = singles.tile([P, JT, S], BF16)
            o2T_sb = singles.tile([P, JT, S], BF16)  # normalized attn out, same rows
            # [sk_p, sk_tile, head, d|ones] -- ones column per head gives the
            # softmax denominator as PSUM row 64 of the PV output
            vaug_sb = singles.tile([P, ET, HL, D + 1], BF16)
            nc.vector.memset(vaug_sb[:, :, :, D:D + 1], 1.0)

            # ---- emission units (each ~8 full-array matmuls + evacuation) ----
            def chain_unit(w_sb, dst, jt, nb, bias_ap):
                ps = ps_mm.tile([P, 512], F32, tag="mm", name=f"mm_{id(w_sb)}_{jt}_{nb}")
                for kt in range(ET):
                    nc.tensor.matmul(
                        ps[:],
                        lhsT=w_sb[:, jt, kt, :],
                        rhs=xT_sb[:, kt, nb * 512:(nb + 1) * 512],
                        start=(kt == 0), stop=(kt == ET - 1),
                    )
                if bias_ap is None:
                    nc.vector.tensor_copy(
                        out=dst[:, jt, nb * 512:(nb + 1) * 512], in_=ps[:])
                else:
                    nc.vector.tensor_scalar_add(
                        dst[:, jt, nb * 512:(nb + 1) * 512], ps[:], bias_ap)

            def v_unit(st):
                ps = ps_mm.tile([P, 512], F32, tag="mm", name=f"mmv_{st}")
                for kt in range(ET):
                    nc.tensor.matmul(
                        ps[:],
                        lhsT=xT_sb[:, kt, st * P:(st + 1) * P],
                        rhs=wv_sb[:, kt, :],
                        start=(kt == 0), stop=(kt == ET - 1),
                    )
                nc.vector.tensor_copy(
                    out=vaug_sb[:, st, :, 0:D],
                    in_=ps[:].rearrange("p (h d) -> p h d", h=HL),
                )

            def pv_unit(h, pT, sqb, pool=None):
                jt0, po = h // 2, (h % 2) * 64
                c0, c1 = sqb * 512, (sqb + 1) * 512
                pl = ps_o if pool is None else pool
                pso = pl.tile([P, 512], F32,
                              tag=("pso" if pl is ps_o else "mm"),
                              name=f"pso_{h}_{sqb}")
                ts = [t for t in range(ET) if t * P < c1]
                for i, t in enumerate(ts):
                    s0 = max(t * P, c0)
                    nc.tensor.matmul(
                        pso[0:D + 1, s0 - c0:512],
                        lhsT=vaug_sb[:, t, h, :],
                        rhs=pT[:, t, s0:c1],
                        start=(i == 0), stop=(i == len(ts) - 1),
                        skip_group_check=True,
                    )
                # normalize: o2T_h[:, c0:c1] = pso[:64] / pso[64]
                rec = rec_pool.tile([P, 512], F32, tag="rec", name=f"rec_{h}_{sqb}")
                nc.vector.tensor_copy(out=rec[:1, :], in_=pso[D:D + 1, :])
                nc.vector.reciprocal_approx_fast(out=rec[:1, :], in_=rec[:1, :])
                bcst = bc_pool.tile([P, 512], F32, tag="bc", name=f"bc_{h}_{sqb}")
                nc.gpsimd.partition_broadcast(bcst[:64, :], rec[:1, :])
                nc.vector.tensor_mul(
                    out=o2T_sb[po:po + 64, jt0, c0:c1],
                    in0=pso[0:D, :], in1=bcst[:64, :],
                )

            def proj_pair(st, split_dma=False):
                ob = out_pool.tile([P, E], BF16, tag="ob", name=f"ob_{st}")
                for eb in range(2):
                    psf = ps_mm.tile([P, 512], F32, tag="mm", name=f"mmp_{st}_{eb}")
                    for kt in range(JT):
                        nc.tensor.matmul(
                            psf[:],
                            lhsT=o2T_sb[:, kt, st * P:(st + 1) * P],
                            rhs=wp_sb[:, kt, eb * 512:(eb + 1) * 512],
                            start=(kt == 0), stop=(kt == JT - 1),
                        )
                    if eb == 0:
                        nc.vector.tensor_copy(out=ob[:, 0:512], in_=psf[:])
                        if split_dma:
                            nc.sync.dma_start(
                                out=out[st * P:(st + 1) * P, 0:512],
                                in_=ob[:, 0:512])
                    else:
                        nc.scalar.copy(out=ob[:, 512:1024], in_=psf[:])
                if split_dma:
                    nc.gpsimd.dma_start(
                        out=out[st * P:(st + 1) * P, 512:1024], in_=ob[:, 512:1024])
                else:
                    eng = nc.sync if st % 2 == 0 else nc.gpsimd
                    eng.dma_start(out=out[st * P:(st + 1) * P, :], in_=ob[:])

            def pair_views(p):
                views = []
                for hh in (2 * p, 2 * p + 1):
                    po = (hh % 2) * 64
                    views.append((
                        qT_sb[po:po + 64, p, :],
                        kT_sb[po:po + 64, p, :],
                        pt_pool.tile([P, ET, S], BF16, tag="pt", name=f"pt_{hh}"),
                    ))
                return views

            def qk_t(views, t):
                lo = t * P
                psls = [ps_l.tile([P, S], F32, tag="psl", name=f"psl_{t}_{j}") for j in range(2)]
                for cb in range(2):
                    c0, c1 = cb * 512, (cb + 1) * 512
                    s0 = max(lo, c0)
                    if s0 >= c1:
                        continue
                    # back-to-back row-half matmuls execute concurrently
                    for (qh, kh, _), psl in zip(views, psls):
                        nc.tensor.matmul(
                            psl[:, s0:c1],
                            lhsT=kh[:, lo:lo + P],
                            rhs=qh[:, s0:c1],
                            start=True, stop=True,
                        )
                for (_, _, pT), psl in zip(views, psls):
                    nc.scalar.activation(
                        out=pT[:, t, lo:S], in_=psl[:, lo:S],
                        func=mybir.ActivationFunctionType.Exp, scale=0.125,
                    )

            def mask_half(views, half):
                # diagonal blocks t=4h..4h+3 in one strided multiply: block t
                # sits at free offset t*(S+P) in the flattened PT tile. Split
                # in halves so PV sqb0 can start right after exp t3.
                t0 = half * 4
                for _, _, pT in views:
                    diag = bass.AP(tensor=pT.tensor,
                                   offset=pT.offset + t0 * (S + P),
                                   ap=[list(pT.ap[0]), [S + P, 4], [1, P]])
                    nc.vector.tensor_mul(
                        out=diag, in0=diag,
                        in1=mask_sb[:, None, :].to_broadcast([P, 4, P]),
                    )

            bias = {}
            if qk_bias:
                for jt in range(JT):
                    bias[("q", jt)] = bqk_sb[:, 0, jt:jt + 1]
                    bias[("k", jt)] = bqk_sb[:, 1, jt:jt + 1]
            else:
                for jt in range(JT):
                    bias[("q", jt)] = None
                    bias[("k", jt)] = None

            def q_unit(jt, nb):
                return lambda: chain_unit(wq_sb, qT_sb, jt, nb, bias[("q", jt)])

            def k_unit(jt, nb):
                return lambda: chain_unit(wk_sb, kT_sb, jt, nb, bias[("k", jt)])

            # ---- phase A: q0+k0 co-accumulated kt-by-kt so the consume
            # rate (8 matmuls per 256KB x-tile) matches the DMA delivery rate;
            # k0 borrows the (idle) ps_o slots ----
            psq = [ps_mm.tile([P, 512], F32, tag="mm", name=f"mmq0_{nb}")
                   for nb in range(2)]
            psk = [ps_o.tile([P, 512], F32, tag="pso", name=f"mmk0_{nb}")
                   for nb in range(2)]
            for kt in range(ET):
                for pss, w_sb in ((psq, wq_sb), (psk, wk_sb)):
                    for nb in range(2):
                        nc.tensor.matmul(
                            pss[nb][:],
                            lhsT=w_sb[:, 0, kt, :],
                            rhs=xT_sb[:, kt, nb * 512:(nb + 1) * 512],
                            start=(kt == 0), stop=(kt == ET - 1),
                        )
            for dst, pss, key in ((qT_sb, psq, ("q", 0)), (kT_sb, psk, ("k", 0))):
                for nb in range(2):
                    if bias[key] is None:
                        nc.vector.tensor_copy(
                            out=dst[:, 0, nb * 512:(nb + 1) * 512], in_=pss[nb][:])
                    else:
                        nc.vector.tensor_scalar_add(
                            dst[:, 0, nb * 512:(nb + 1) * 512], pss[nb][:], bias[key])

            # ---- attention weave: fillers between QK t-steps; each pair's
            # sqb0 PV runs inside its own window (mask halves), sqb1 early in
            # the next window ----
            def pv(h, views, sqb, pool=None):
                return lambda: pv_unit(h, views[h % 2][2], sqb, pool)

            views0 = pair_views(0)
            f0 = {0: [lambda: v_unit(0)], 1: [lambda: v_unit(1)],
                  2: [lambda: v_unit(2)], 3: [lambda: v_unit(3)],
                  4: [q_unit(1, 0), q_unit(1, 1)],
                  5: [k_unit(1, 0), k_unit(1, 1)],
                  6: [pv(0, views0, 0)], 7: [pv(1, views0, 0)]}
            for t in range(ET):
                qk_t(views0, t)
                if t == 3:
                    mask_half(views0, 0)
                for u in f0.get(t, []):
                    u()
            mask_half(views0, 1)

            views1 = pair_views(1)
            f1 = {0: [lambda: v_unit(4)], 1: [lambda: v_unit(5)],
                  2: [lambda: v_unit(6)], 3: [lambda: v_unit(7)],
                  4: [q_unit(2, 0)], 5: [q_unit(2, 1)],
                  6: [pv(2, views1, 0), k_unit(2, 0)],
                  7: [pv(3, views1, 0), k_unit(2, 1)]}
            for t in range(ET):
                qk_t(views1, t)
                if t == 3:
                    mask_half(views1, 0)
                for u in f1.get(t, []):
                    u()
            mask_half(views1, 1)

            views2 = pair_views(2)
            f2 = {0: [pv(0, views0, 1)], 1: [pv(1, views0, 1)],
                  2: [pv(2, views1, 1)], 3: [pv(3, views1, 1)],
                  4: [q_unit(3, 0)], 5: [q_unit(3, 1)],
                  6: [pv(4, views2, 0), k_unit(3, 0)],
                  7: [pv(5, views2, 0), k_unit(3, 1)]}
            for t in range(ET):
                qk_t(views2, t)
                if t == 3:
                    mask_half(views2, 0)
                for u in f2.get(t, []):
                    u()
            mask_half(views2, 1)

            views3 = pair_views(3)
            f3 = {0: [pv(4, views2, 1)], 1: [pv(5, views2, 1)],
                  4: [pv(6, views3, 0)], 5: [pv(7, views3, 0)],
                  7: [lambda: proj_pair(0)]}
            for t in range(ET):
                qk_t(views3, t)
                if t == 3:
                    mask_half(views3, 0)
                for u in f3.get(t, []):
                    u()
            mask_half(views3, 1)

            # ---- endgame: last sqb1 PVs woven into the projection ----
            pv_unit(6, views3[0][2], 1)
            proj_pair(1)
            pv_unit(7, views3[1][2], 1)
            for st in range(2, ET):
                proj_pair(st, split_dma=(st >= ET - 2))

    nc.compile()
    return nc


def make_in_maps(x, W_attn, b_attn, W_proj, b_proj):
    bf16 = ml_dtypes.bfloat16
    x = np.asarray(x, dtype=np.float32)
    W_attn = np.asarray(W_attn, dtype=np.float32)
    b_attn = np.asarray(b_attn, dtype=np.float32)
    W_proj = np.asarray(W_proj, dtype=np.float32)
    qk_bias = bool(b_attn[:2 * E].any())
    in_maps = []
    for i in range(NCORES):
        b, g = i // 2, i % 2
        j0 = g * JL
        wq_s = W_attn[:, j0:j0 + JL]
        wk_s = W_attn[:, E + j0:E + j0 + JL]
        wv_s = W_attn[:, 2 * E + j0:2 * E + j0 + JL]
        wp_s = W_proj[j0:j0 + JL, :]
        m = {
            # [p, kt, s]: partition-contiguous x^T
            "xT": np.ascontiguousarray(
                x[b].T.reshape(ET, P, S).transpose(1, 0, 2)
            ).astype(bf16).reshape(P, ET * S),
            # [p, jt, kt, j]
            "wq": np.ascontiguousarray(
                wq_s.reshape(ET, P, JT, P).transpose(1, 2, 0, 3)
            ).astype(bf16).reshape(P, JT * ET * P),
            "wk": np.ascontiguousarray(
                wk_s.reshape(ET, P, JT, P).transpose(1, 2, 0, 3)
            ).astype(bf16).reshape(P, JT * ET * P),
            # [p, kt, j]
            "wv": np.ascontiguousarray(
                wv_s.reshape(ET, P, JL).transpose(1, 0, 2)
            ).astype(bf16).reshape(P, ET * JL),
            # [p, jt, e]
            "wp": np.ascontiguousarray(
                wp_s.reshape(JT, P, E).transpose(1, 0, 2)
            ).astype(bf16).reshape(P, JT * E),
        }
        if qk_bias:
            bq = b_attn[j0:j0 + JL].reshape(JT, P).T
            bk = b_attn[E + j0:E + j0 + JL].reshape(JT, P).T
            m["bqk"] = np.ascontiguousarray(
                np.stack([bq, bk], axis=1)).reshape(P, 2 * JT).astype(np.float32)
        in_maps.append(m)
    return in_maps


def kernel(x, W_attn, b_attn, W_proj, b_proj):
    global _NC_CACHE
    x = np.asarray(x, dtype=np.float32)
    W_attn = np.asarray(W_attn, dtype=np.float32)
    b_attn = np.asarray(b_attn, dtype=np.float32)
    W_proj = np.asarray(W_proj, dtype=np.float32)
    b_proj = np.asarray(b_proj, dtype=np.float32)

    qk_bias = bool(b_attn[:2 * E].any())
    if qk_bias not in _NC_CACHE:
        _NC_CACHE[qk_bias] = build_nc(qk_bias)
    nc = _NC_CACHE[qk_bias]

    in_maps = make_in_maps(x, W_attn, b_attn, W_proj, b_proj)
    res = run_bass_kernel_spmd(nc, in_maps, core_ids=list(range(NCORES)))

    # host unshard: sum the two head-group partials + exact bias corrections
    bias_row = b_proj.copy()
    for g in range(2):
        j0 = g * JL
        bv = b_attn[2 * E + j0:2 * E + j0 + JL].astype(np.float32)
        bias_row += bv @ W_proj[j0:j0 + JL, :].astype(np.float32)

    full = np.empty((B, S, E), np.float32)
    for b in range(B):
        full[b] = (res.results[2 * b]["out"].astype(np.float32)
                   + res.results[2 * b + 1]["out"].astype(np.float32)
                   + bias_row[None, :])
    return full
